# revision 22
# baseline (speedup 1.0000x reference)
"""Tensor-parallel GQA attention forward for one TRN2 chip (8 NeuronCores).

Head-parallel strategy (v2):
  - host passes full xT (d-major, bf16) to every core; each core projects ONLY
    its own 4 q-heads + 1 kv-head with the weight tile stationary, producing
    qT/kT directly in attention layout (no receiver transposes, no projection
    collectives at all)
  - RoPE applied in transposed orientation (head_dim on partitions,
    de-interleaved [ev|od]); cos/sin tables arrive pre-transposed+replicated
  - chunk-pipelined schedule: attention for q-chunk c runs while the
    projection matmuls for chunk c+1 are interleaved into its exp-bound PE
    stalls (softmax exp on the scalar engine is the per-chunk bottleneck)
  - causal trimming: score matmuls + exp skip the below-diagonal-dead columns
    of diagonal k-tiles; probs outside get memset to 0 for the full-width PV
  - softmax denominators ride as a 65th..128th column of ones in the PV
    matmul; one AllToAll per head-pair flips attnT to sequence-sharded; each
    core then computes its 256-row slice of the output projection vs full wo
  - compute dtype bf16 (fp32 PSUM accumulation), output fp32
"""

import numpy as np

NC_CORES = 8
SEQ = 2048
DIM = 2048
HD = 64            # head dim
SC = SEQ // NC_CORES   # 256: sequence rows per core (output shard)
CH = 512           # q-chunk width for attention
NCH = SEQ // CH    # 4
KT = SEQ // 128    # 16 k-tiles
DT = DIM // 128    # 16 d-tiles
WCOLS = 384        # per-core weight cols: q pair0 (128) | q pair1 (128) | k 64 | v 64

_CACHE = {}


def _build_nc():
    import concourse.bass as bass
    import concourse.mybir as mybir
    import concourse.tile as tile
    from concourse import bacc
    from concourse.masks import make_identity

    BF = mybir.dt.bfloat16
    F32 = mybir.dt.float32
    MUL = mybir.AluOpType.mult
    ADD = mybir.AluOpType.add
    SUB = mybir.AluOpType.subtract

    nc = bacc.Bacc("TRN2", target_bir_lowering=False, debug=False,
                   num_devices=NC_CORES)

    xT = nc.dram_tensor("xT", [DIM, SEQ], BF, kind="ExternalInput")
    wqkv = nc.dram_tensor("wqkv", [DIM, WCOLS], BF, kind="ExternalInput")
    wo = nc.dram_tensor("wo", [DIM, DIM], BF, kind="ExternalInput")
    cosr = nc.dram_tensor("cosr", [128, SEQ], BF, kind="ExternalInput")
    sinr = nc.dram_tensor("sinr", [128, SEQ], BF, kind="ExternalInput")
    out = nc.dram_tensor("out", [SC, DIM], F32, kind="ExternalOutput")

    groups = [list(range(NC_CORES))]

    with tile.TileContext(nc) as tc:
        # DRAM bounce buffers for the two attnT AllToAlls
        a2a_in0, _ = tc.tile([NC_CORES, 128, SC], BF,
                             space=bass.MemorySpace.DRAM, name="a2a_in0")
        a2a_out0, _ = tc.tile([NC_CORES, 128, SC], BF,
                              space=bass.MemorySpace.DRAM,
                              addr_space="Shared", name="a2a_out0")
        a2a_in1, _ = tc.tile([NC_CORES, 128, SC], BF,
                             space=bass.MemorySpace.DRAM, name="a2a_in1")
        a2a_out1, _ = tc.tile([NC_CORES, 128, SC], BF,
                              space=bass.MemorySpace.DRAM,
                              addr_space="Shared", name="a2a_out1")

        with tc.tile_pool(name="persist", bufs=1) as pp, \
             tc.tile_pool(name="work", bufs=2) as wp, \
             tc.tile_pool(name="psum", bufs=2, space="PSUM") as psp:

            ident = pp.tile([128, 128], BF, name="ident")
            make_identity(nc, ident[:])

            # triangle causal pattern, 1 where q-col >= k-row, for both heads
            patd = pp.tile([128, 2, 128], BF, name="patd")
            nc.gpsimd.memset(patd[:], 1.0)
            for h in range(2):
                nc.gpsimd.affine_select(
                    out=patd[:, h, :], in_=patd[:, h, :],
                    compare_op=mybir.AluOpType.is_ge, fill=0.0,
                    base=0, channel_multiplier=-1, pattern=[[1, 128]],
                )

            # prepay the exp ACT-table load (~2.7us) while DMAs stream
            warmup = pp.tile([1, 1], BF, name="warmup")
            nc.scalar.activation(warmup[:], ident[0:1, 0:1],
                                 mybir.ActivationFunctionType.Exp, scale=1.0)

            cos_sb = pp.tile([128, SEQ], BF, name="cos_sb")
            sin_sb = pp.tile([128, SEQ], BF, name="sin_sb")
            nc.gpsimd.dma_start(cos_sb[:], cosr[:])
            nc.gpsimd.dma_start(sin_sb[:], sinr[:])

            # weights on the gpsimd DMA queue so they stream concurrently
            # with the xT chunks on the sync queue
            wqkv_sb = pp.tile([128, DT, WCOLS], BF, name="wqkv_sb")
            nc.gpsimd.dma_start(wqkv_sb[:],
                                wqkv[:].rearrange("(t p) m -> p t m", p=128))

            # xT chunk stream (issue all DMAs up front on the sync queue so wo
            # streams strictly after; pool bufs gate the actual transfers)
            xt_tiles = []
            for c in range(NCH):
                xt = wp.tile([128, DT, CH], BF, tag="xT", bufs=2,
                             name=f"xt{c}")
                nc.sync.dma_start(
                    xt[:], xT[:, CH * c:CH * c + CH]
                    .rearrange("(t p) s -> p t s", p=128))
                xt_tiles.append(xt)

            # wo groups are DMA'd later with data-dependency anchors so the
            # Tile scheduler cannot hoist them ahead of the xT stream
            wo_sb = pp.tile([128, DT, DIM], BF, name="wo_sb")

            # per-chunk kT / v tiles: separate tiles keep attention reads on
            # older chunks from false-depending on the newest chunk's writes
            kT_c = [pp.tile([128, CH], BF, name=f"kT{c}") for c in range(NCH)]
            v_c = [pp.tile([128, 4, 2 * HD], BF, name=f"v{c}")
                   for c in range(NCH)]
            for c in range(NCH):
                nc.gpsimd.memset(v_c[c][:, :, HD:2 * HD], 1.0)
            attnT = pp.tile([128, 2, SEQ], BF, name="attnT")

            qT_t = {}   # (chunk mod 2 handled by pool bufs) -> per-pair tiles

            def rope_apply(dst, src, nrows, sl):
                # dst = src*cos + rotate_half(src)*(+-sin); src is PSUM fp32
                # with rows in [ev(32)|od(32)] blocks; the ev-rows of sin_sb
                # carry a negated table so all tensor_tensor base partitions
                # align (NCC_IBIR297: 2-input SBUF ops need equal bases).
                rh = wp.tile([128, CH], F32, tag="rh", bufs=2, name="rh")
                for b in range(nrows // 64):
                    nc.vector.tensor_copy(rh[64 * b:64 * b + 32, :],
                                          src[64 * b + 32:64 * b + 64, :])
                    nc.vector.tensor_copy(rh[64 * b + 32:64 * b + 64, :],
                                          src[64 * b:64 * b + 32, :])
                t1 = wp.tile([128, CH], F32, tag="rp1", bufs=2, name="t1")
                nc.vector.tensor_tensor(t1[0:nrows, :], src[0:nrows, :],
                                        cos_sb[0:nrows, sl], MUL)
                nc.vector.tensor_tensor(rh[0:nrows, :], rh[0:nrows, :],
                                        sin_sb[0:nrows, sl], MUL)
                nc.vector.tensor_tensor(dst, t1[0:nrows, :], rh[0:nrows, :],
                                        ADD)

            def make_proj_tasks(c):
                """Projection of chunk c: (tasks, post) lists of closures.

                `tasks` are safe to interleave into an attention k-tile loop
                (each emits at most one PE op whose waits resolve on other
                engines); `post` (the v PE-transposes, which cycle the pv
                psum ring shared with open PV accumulators) may only run at
                the chunk drain.
                """
                xt = xt_tiles[c]
                sl = slice(CH * c, CH * c + CH)
                tasks = []
                pjq = psp.tile([128, 2, CH], F32, tag="pj", bufs=1,
                               name=f"pjq{c}")

                def q_mm(b, dt):
                    def f():
                        nc.tensor.matmul(
                            pjq[:, b, :], wqkv_sb[:, dt, 128 * b:128 * b + 128],
                            xt[:, dt, :], start=(dt == 0), stop=(dt == DT - 1))
                    return f

                def q_rope(b):
                    def f():
                        # bufs=4: pair-major attention needs every chunk's qT
                        # alive through the pair-1 pass
                        q = wp.tile([128, CH], BF, tag=f"qT{b}", bufs=4,
                                    name=f"q{b}_{c}")
                        qT_t[(c, b)] = q
                        rope_apply(q[:], pjq[:, b, :], 128, sl)
                    return f
                for b in range(2):
                    for dt in range(DT):
                        tasks.append(q_mm(b, dt))
                    tasks.append(q_rope(b))

                pjk = psp.tile([128, CH], F32, tag="pj", bufs=1,
                               name=f"pjk{c}")

                def kv_mm(dt):
                    def f():
                        nc.tensor.matmul(
                            pjk[:], wqkv_sb[:, dt, 256:384], xt[:, dt, :],
                            start=(dt == 0), stop=(dt == DT - 1))
                    return f
                for dt in range(DT):
                    tasks.append(kv_mm(dt))

                def kv_fin():
                    # rope k (rows 0:64) into kT, duplicate to rows 64:128
                    rope_apply(kT_c[c][0:64, :], pjk[:], 64, sl)
                    nc.gpsimd.tensor_copy(kT_c[c][64:128, :], kT_c[c][0:64, :])
                    # v: psum rows 64:128 -> staging -> PE transpose -> v_sb
                    vst = wp.tile([64, CH], BF, tag="vst", bufs=2, name="vst")
                    nc.vector.tensor_copy(vst[:], pjk[64:128, :])
                    qT_t[("vst", c)] = vst
                tasks.append(kv_fin)

                def v_tr(g):
                    def f():
                        vst = qT_t[("vst", c)]
                        tp = psp.tile([128, 128], BF, tag="pv", bufs=2,
                                      name="tp")
                        nc.tensor.transpose(
                            tp[:, 0:64], vst[:, 128 * g:128 * g + 128],
                            ident[0:64, 0:64])
                        nc.vector.tensor_copy(v_c[c][:, g, 0:HD],
                                              tp[:, 0:64])
                    return f
                post = [v_tr(g) for g in range(4)]
                return tasks, post

            def attention(c, p, filler):
                nkt = 4 * c + 4
                qTc = qT_t[(c, p)]
                qsl = slice(CH * c, CH * c + CH)
                pso0 = psp.tile([128, CH], F32, tag="pv", bufs=2, name="pso0")
                pso1 = psp.tile([128, CH], F32, tag="pv", bufs=2, name="pso1")
                pend = []   # (kt, ep, off) awaiting PV
                for kt in range(nkt):
                    kTk = kT_c[kt // 4]
                    ks = slice(128 * (kt % 4), 128 * (kt % 4) + 128)
                    dt_ = kt - 4 * c
                    off = 128 * dt_ if dt_ >= 0 else 0
                    sp = psp.tile([128, 2, CH], F32, tag="sp", bufs=2,
                                  name="sp")
                    nc.tensor.matmul(sp[:, 0, off:CH], kTk[0:64, ks],
                                     qTc[0:64, off:CH], start=True, stop=True)
                    nc.tensor.matmul(sp[:, 1, off:CH], kTk[64:128, ks],
                                     qTc[64:128, off:CH], start=True,
                                     stop=True)
                    ep = wp.tile([128, 2, CH], BF, tag="ep", bufs=3, name="ep")
                    nc.scalar.activation(ep[:, :, off:CH], sp[:, :, off:CH],
                                         mybir.ActivationFunctionType.Exp,
                                         scale=0.125)
                    if dt_ >= 0:
                        nc.vector.tensor_tensor(
                            ep[:, :, off:off + 128], ep[:, :, off:off + 128],
                            patd[:], MUL)
                    # drain previous k-tile's PV now (exp of this tile runs on
                    # ACT meanwhile), then interleave filler PE work.  PV is
                    # column-trimmed like the scores: columns below a diagonal
                    # tile's band take no contribution from it.
                    for (pkt, pep, poff) in pend:
                        vv = v_c[pkt // 4][:, pkt % 4, :]
                        nc.tensor.matmul(pso0[:, poff:CH], vv,
                                         pep[:, 0, poff:CH], start=(pkt == 0),
                                         stop=False)
                        nc.tensor.matmul(pso1[:, poff:CH], vv,
                                         pep[:, 1, poff:CH], start=(pkt == 0),
                                         stop=False)
                    pend = [(kt, ep, off)]
                    if filler:
                        filler.pop(0)()
                        if len(filler) % 2 == 0 and filler:
                            filler.pop(0)()
                for (pkt, pep, poff) in pend:
                    vv = v_c[pkt // 4][:, pkt % 4, :]
                    nc.tensor.matmul(pso0[:, poff:CH], vv,
                                     pep[:, 0, poff:CH], start=(pkt == 0),
                                     stop=True)
                    nc.tensor.matmul(pso1[:, poff:CH], vv,
                                     pep[:, 1, poff:CH], start=(pkt == 0),
                                     stop=True)
                for h, pso in ((0, pso0), (1, pso1)):
                    bc = wp.tile([64, CH], F32, tag="bcast", bufs=2, name="bc")
                    nc.vector.tensor_copy(bc[:], pso[HD:2 * HD, :])
                    rc = wp.tile([64, CH], F32, tag="rcp", bufs=2, name="rc")
                    nc.vector.reciprocal_approx_fast(out=rc[:], in_=bc[:])
                    nc.vector.tensor_tensor(
                        attnT[64 * h:64 * h + 64, p, qsl],
                        pso[0:HD, :], rc[:], MUL)

            # ---------------- output projection helpers ----------------
            a2a_sb0 = pp.tile([128, NC_CORES, SC], BF, name="a2a_sb0")
            a2a_sb1 = pp.tile([128, NC_CORES, SC], BF, name="a2a_sb1")
            partials = pp.tile([128, 2 * NCH, CH], BF, name="partials")
            evens = [2 * src for src in range(NC_CORES)]
            odds = [2 * src + 1 for src in range(NC_CORES)]
            chunks = [(qt, nch) for qt in range(2) for nch in range(NCH)]

            def op_mm(psf, qt, nsl, g, start, stop):
                a_ap = (a2a_sb0[:, g // 2, 128 * qt:128 * qt + 128]
                        if g % 2 == 0
                        else a2a_sb1[:, g // 2, 128 * qt:128 * qt + 128])
                nc.tensor.matmul(psf[:], a_ap, wo_sb[:, g, nsl],
                                 start=start, stop=stop)

            ev_psf = {}

            def ev_group(i8, qt, nch):
                nsl = slice(CH * nch, CH * nch + CH)

                def mk(i, g):
                    def f():
                        if i == 0:
                            ev_psf[i8] = psp.tile([128, CH], F32, tag="pj",
                                                  bufs=1, name=f"psfE{i8}")
                        op_mm(ev_psf[i8], qt, nsl, g, i == 0,
                              i == NC_CORES - 1)
                    return f
                fs = [mk(i, g) for i, g in enumerate(evens)]

                def fin():
                    nc.vector.tensor_copy(partials[:, i8, :], ev_psf[i8][:])
                fs.append(fin)
                return fs

            # ---------------- main pipeline (pair-major) ----------------
            tasks, post = make_proj_tasks(0)
            for t in tasks + post:
                t()
            for c in range(NCH):
                if c + 1 < NCH:
                    filler, post = make_proj_tasks(c + 1)
                else:
                    filler, post = [], []
                attention(c, 0, filler)
                for dst in (2 * c, 2 * c + 1):
                    nc.sync.dma_start(a2a_in0[dst, :, :],
                                      attnT[:, 0, SC * dst:SC * dst + SC])
                # anchored wo prefetch: the 1-element write makes the DMA
                # wait until this point instead of competing with xT early
                nc.vector.tensor_copy(wo_sb[0:1, 4 * c, 0:1],
                                      attnT[0:1, 0, 0:1])
                nc.sync.dma_start(
                    wo_sb[:, 4 * c:4 * c + 4, :],
                    wo[512 * c:512 * c + 512, :]
                    .rearrange("(t p) n -> p t n", p=128))
                for t in filler + post:
                    t()
            nc.gpsimd.collective_compute(
                "AllToAll", mybir.AluOpType.bypass,
                replica_groups=groups, ins=[a2a_in0.opt()],
                outs=[a2a_out0.opt()],
            )
            for src in range(NC_CORES):
                nc.sync.dma_start(a2a_sb0[:, src, :], a2a_out0[src, :, :])

            ev_tasks = []
            for i8, (qt, nch) in enumerate(chunks):
                ev_tasks += ev_group(i8, qt, nch)

            for c in range(NCH):
                # interleave the evens half of the output projection into the
                # late pair-1 chunks (a2a_sb0 has long landed by then)
                attention(c, 1, ev_tasks if c >= 2 else [])
                for dst in (2 * c, 2 * c + 1):
                    nc.sync.dma_start(a2a_in1[dst, :, :],
                                      attnT[:, 1, SC * dst:SC * dst + SC])
            nc.gpsimd.collective_compute(
                "AllToAll", mybir.AluOpType.bypass,
                replica_groups=groups, ins=[a2a_in1.opt()],
                outs=[a2a_out1.opt()],
            )
            for src in range(NC_CORES):
                nc.sync.dma_start(a2a_sb1[:, src, :], a2a_out1[src, :, :])
            for t in ev_tasks:
                t()

            for i8, (qt, nch) in enumerate(chunks):
                psf = psp.tile([128, CH], F32, tag="sp", bufs=2, name="psfO")
                nsl = slice(CH * nch, CH * nch + CH)
                for i, g in enumerate(odds):
                    op_mm(psf, qt, nsl, g, i == 0, i == NC_CORES - 1)
                osb = wp.tile([128, CH], F32, tag="osb", bufs=2, name="osb")
                nc.vector.tensor_tensor(osb[:], psf[:], partials[:, i8, :],
                                        ADD)
                nc.sync.dma_start(out[128 * qt:128 * qt + 128, nsl], osb[:])

    nc.finalize()
    return nc


def _get_nc():
    if "nc" not in _CACHE:
        _CACHE["nc"] = _build_nc()
    return _CACHE["nc"]


_PERM = np.concatenate([np.arange(0, HD, 2), np.arange(1, HD, 2)])  # de-interleave


def _shard(inputs):
    import ml_dtypes
    BF = ml_dtypes.bfloat16
    x = np.asarray(inputs["x"][0], dtype=np.float32)                 # [S, D]
    xT = np.ascontiguousarray(x.T.astype(BF))                        # [D, S]
    wq = np.asarray(inputs["wq"], dtype=np.float32)
    wk = np.asarray(inputs["wk"], dtype=np.float32)
    wv = np.asarray(inputs["wv"], dtype=np.float32)
    wo = np.ascontiguousarray(np.asarray(inputs["wo"]).astype(BF))
    cos = np.asarray(inputs["freqs_cos"], dtype=np.float32)          # [S, 32]
    sin = np.asarray(inputs["freqs_sin"], dtype=np.float32)
    cosr = np.ascontiguousarray(np.tile(cos.T, (4, 1)).astype(BF))   # [128, S]
    # ev-rows get -sin so rotate_half(x)*sinr lands with the right signs
    sinr = np.ascontiguousarray(
        np.concatenate([-sin.T, sin.T, -sin.T, sin.T], axis=0).astype(BF))
    wq_p = wq.reshape(DIM, 32, HD)[:, :, _PERM]                      # [D,32,64]
    wk_p = wk.reshape(DIM, 8, HD)[:, :, _PERM]
    in_maps = []
    for c in range(NC_CORES):
        q0 = wq_p[:, 4 * c:4 * c + 2, :].reshape(DIM, 128)
        q1 = wq_p[:, 4 * c + 2:4 * c + 4, :].reshape(DIM, 128)
        kc = wk_p[:, c, :]
        vc = wv[:, HD * c:HD * c + HD]
        wqkv = np.ascontiguousarray(
            np.concatenate([q0, q1, kc, vc], axis=1).astype(BF))
        in_maps.append({
            "xT": xT,
            "wqkv": wqkv,
            "wo": wo,
            "cosr": cosr,
            "sinr": sinr,
        })
    return in_maps


def kernel(**inputs):
    from concourse.bass_utils import run_bass_kernel_spmd

    nc = _get_nc()
    in_maps = _shard(inputs)
    res = run_bass_kernel_spmd(nc, in_maps, core_ids=list(range(NC_CORES)))
    out = np.concatenate([res.results[c]["out"] for c in range(NC_CORES)],
                         axis=0)
    return out[None].astype(np.float32)


# revision 27
# speedup vs baseline: 1.2280x; 1.2280x over previous
"""Tensor-parallel GQA attention forward for one TRN2 chip (8 NeuronCores).

Head-parallel strategy (v2):
  - host passes full xT (d-major, bf16) to every core; each core projects ONLY
    its own 4 q-heads + 1 kv-head with the weight tile stationary, producing
    qT/kT directly in attention layout (no receiver transposes, no projection
    collectives at all)
  - RoPE applied in transposed orientation (head_dim on partitions,
    de-interleaved [ev|od]); cos/sin tables arrive pre-transposed+replicated
  - chunk-pipelined schedule: attention for q-chunk c runs while the
    projection matmuls for chunk c+1 are interleaved into its exp-bound PE
    stalls (softmax exp on the scalar engine is the per-chunk bottleneck)
  - causal trimming: score matmuls + exp skip the below-diagonal-dead columns
    of diagonal k-tiles; probs outside get memset to 0 for the full-width PV
  - softmax denominators ride as a 65th..128th column of ones in the PV
    matmul; one AllToAll per head-pair flips attnT to sequence-sharded; each
    core then computes its 256-row slice of the output projection vs full wo
  - compute dtype bf16 (fp32 PSUM accumulation), output fp32
"""

import numpy as np

NC_CORES = 8
SEQ = 2048
DIM = 2048
HD = 64            # head dim
SC = SEQ // NC_CORES   # 256: sequence rows per core (output shard)
CH = 512           # q-chunk width for attention
NCH = SEQ // CH    # 4
KT = SEQ // 128    # 16 k-tiles
DT = DIM // 128    # 16 d-tiles
WCOLS = 384        # per-core weight cols: q pair0 (128) | q pair1 (128) | k 64 | v 64

_CACHE = {}


def _build_nc():
    import concourse.bass as bass
    import concourse.mybir as mybir
    import concourse.tile as tile
    from concourse import bacc
    from concourse.masks import make_identity

    BF = mybir.dt.bfloat16
    F32 = mybir.dt.float32
    MUL = mybir.AluOpType.mult
    ADD = mybir.AluOpType.add
    SUB = mybir.AluOpType.subtract

    nc = bacc.Bacc("TRN2", target_bir_lowering=False, debug=False,
                   num_devices=NC_CORES)

    # host pre-tiles everything into SBUF layout so DMAs are contiguous
    xT = nc.dram_tensor("xT", [NCH, 128, DT, CH], BF, kind="ExternalInput")
    wqkv = nc.dram_tensor("wqkv", [128, DT, WCOLS], BF, kind="ExternalInput")
    wo = nc.dram_tensor("wo", [128, DT, DIM], BF, kind="ExternalInput")
    cosr = nc.dram_tensor("cosr", [128, SEQ], BF, kind="ExternalInput")
    sinr = nc.dram_tensor("sinr", [128, SEQ], BF, kind="ExternalInput")
    out = nc.dram_tensor("out", [SC, DIM], F32, kind="ExternalOutput")

    groups = [list(range(NC_CORES))]

    with tile.TileContext(nc) as tc:
        # DRAM bounce buffers for the two attnT AllToAlls
        a2a_in0, _ = tc.tile([NC_CORES, 128, SC], BF,
                             space=bass.MemorySpace.DRAM, name="a2a_in0")
        a2a_out0, _ = tc.tile([NC_CORES, 128, SC], BF,
                              space=bass.MemorySpace.DRAM,
                              addr_space="Shared", name="a2a_out0")
        a2a_in1, _ = tc.tile([NC_CORES, 128, SC], BF,
                             space=bass.MemorySpace.DRAM, name="a2a_in1")
        a2a_out1, _ = tc.tile([NC_CORES, 128, SC], BF,
                              space=bass.MemorySpace.DRAM,
                              addr_space="Shared", name="a2a_out1")

        with tc.tile_pool(name="persist", bufs=1) as pp, \
             tc.tile_pool(name="work", bufs=2) as wp, \
             tc.tile_pool(name="psum", bufs=2, space="PSUM") as psp:

            ident = pp.tile([128, 128], BF, name="ident")
            make_identity(nc, ident[:])

            # triangle causal pattern, 1 where q-col >= k-row, for both heads
            patd = pp.tile([128, 2, 128], BF, name="patd")
            nc.gpsimd.memset(patd[:], 1.0)
            for h in range(2):
                nc.gpsimd.affine_select(
                    out=patd[:, h, :], in_=patd[:, h, :],
                    compare_op=mybir.AluOpType.is_ge, fill=0.0,
                    base=0, channel_multiplier=-1, pattern=[[1, 128]],
                )

            # prepay the exp ACT-table load (~2.7us) while DMAs stream
            warmup = pp.tile([1, 1], BF, name="warmup")
            nc.scalar.activation(warmup[:], ident[0:1, 0:1],
                                 mybir.ActivationFunctionType.Exp, scale=1.0)

            cos_sb = pp.tile([128, SEQ], BF, name="cos_sb")
            sin_sb = pp.tile([128, SEQ], BF, name="sin_sb")
            nc.gpsimd.dma_start(cos_sb[:], cosr[:])
            nc.gpsimd.dma_start(sin_sb[:], sinr[:])

            # weights on the gpsimd DMA queue so they stream concurrently
            # with the xT chunks on the sync queue
            wqkv_sb = pp.tile([128, DT, WCOLS], BF, name="wqkv_sb")
            nc.gpsimd.dma_start(wqkv_sb[:], wqkv[:])

            # xT chunk stream (issue all DMAs up front on the sync queue;
            # pool bufs gate the actual transfers)
            xt_tiles = []
            for c in range(NCH):
                xt = wp.tile([128, DT, CH], BF, tag="xT", bufs=2,
                             name=f"xt{c}")
                nc.sync.dma_start(xt[:], xT[c, :, :, :])
                xt_tiles.append(xt)

            # wo groups are DMA'd later with data-dependency anchors so the
            # Tile scheduler cannot hoist them ahead of the xT stream
            wo_sb = pp.tile([128, DT, DIM], BF, name="wo_sb")

            # per-chunk kT / v tiles: separate tiles keep attention reads on
            # older chunks from false-depending on the newest chunk's writes
            kT_c = [pp.tile([128, CH], BF, name=f"kT{c}") for c in range(NCH)]
            v_c = [pp.tile([128, 4, 2 * HD], BF, name=f"v{c}")
                   for c in range(NCH)]
            for c in range(NCH):
                nc.gpsimd.memset(v_c[c][:, :, HD:2 * HD], 1.0)
            attnT = pp.tile([128, 2, SEQ], BF, name="attnT")

            qT_t = {}   # (chunk mod 2 handled by pool bufs) -> per-pair tiles

            def rope_apply(dst, src, nrows, sl):
                # dst = src*cos + rotate_half(src)*(+-sin); src is PSUM fp32
                # with rows in [ev(32)|od(32)] blocks; the ev-rows of sin_sb
                # carry a negated table so all tensor_tensor base partitions
                # align (NCC_IBIR297: 2-input SBUF ops need equal bases).
                rh = wp.tile([128, CH], F32, tag="rh", bufs=2, name="rh")
                for b in range(nrows // 64):
                    nc.vector.tensor_copy(rh[64 * b:64 * b + 32, :],
                                          src[64 * b + 32:64 * b + 64, :])
                    nc.vector.tensor_copy(rh[64 * b + 32:64 * b + 64, :],
                                          src[64 * b:64 * b + 32, :])
                t1 = wp.tile([128, CH], F32, tag="rp1", bufs=2, name="t1")
                nc.vector.tensor_tensor(t1[0:nrows, :], src[0:nrows, :],
                                        cos_sb[0:nrows, sl], MUL)
                nc.vector.tensor_tensor(rh[0:nrows, :], rh[0:nrows, :],
                                        sin_sb[0:nrows, sl], MUL)
                nc.vector.tensor_tensor(dst, t1[0:nrows, :], rh[0:nrows, :],
                                        ADD)

            def make_proj_tasks(c):
                """Projection of chunk c: (tasks, post) lists of closures.

                `tasks` are safe to interleave into an attention k-tile loop
                (each emits at most one PE op whose waits resolve on other
                engines); `post` (the v PE-transposes, which cycle the pv
                psum ring shared with open PV accumulators) may only run at
                the chunk drain.
                """
                xt = xt_tiles[c]
                sl = slice(CH * c, CH * c + CH)
                tasks = []
                pjq = psp.tile([128, 2, CH], F32, tag="pj", bufs=1,
                               name=f"pjq{c}")

                def q_mm(b, dt):
                    def f():
                        nc.tensor.matmul(
                            pjq[:, b, :], wqkv_sb[:, dt, 128 * b:128 * b + 128],
                            xt[:, dt, :], start=(dt == 0), stop=(dt == DT - 1))
                    return f

                def q_rope(b):
                    def f():
                        # bufs=4: pair-major attention needs every chunk's qT
                        # alive through the pair-1 pass
                        q = wp.tile([128, CH], BF, tag=f"qT{b}", bufs=4,
                                    name=f"q{b}_{c}")
                        qT_t[(c, b)] = q
                        rope_apply(q[:], pjq[:, b, :], 128, sl)
                    return f
                for b in range(2):
                    for dt in range(DT):
                        tasks.append(q_mm(b, dt))
                    tasks.append(q_rope(b))

                pjk = psp.tile([128, CH], F32, tag="pj", bufs=1,
                               name=f"pjk{c}")

                def kv_mm(dt):
                    def f():
                        nc.tensor.matmul(
                            pjk[:], wqkv_sb[:, dt, 256:384], xt[:, dt, :],
                            start=(dt == 0), stop=(dt == DT - 1))
                    return f
                for dt in range(DT):
                    tasks.append(kv_mm(dt))

                def kv_fin():
                    # rope k (rows 0:64) into kT, duplicate to rows 64:128
                    rope_apply(kT_c[c][0:64, :], pjk[:], 64, sl)
                    nc.gpsimd.tensor_copy(kT_c[c][64:128, :], kT_c[c][0:64, :])
                    # v: psum rows 64:128 -> staging -> PE transpose -> v_sb
                    vst = wp.tile([64, CH], BF, tag="vst", bufs=2, name="vst")
                    nc.vector.tensor_copy(vst[:], pjk[64:128, :])
                    qT_t[("vst", c)] = vst
                tasks.append(kv_fin)

                def v_tr(g):
                    def f():
                        vst = qT_t[("vst", c)]
                        tp = psp.tile([128, 128], BF, tag="pv", bufs=2,
                                      name="tp")
                        nc.tensor.transpose(
                            tp[:, 0:64], vst[:, 128 * g:128 * g + 128],
                            ident[0:64, 0:64])
                        nc.vector.tensor_copy(v_c[c][:, g, 0:HD],
                                              tp[:, 0:64])
                    return f
                post = [v_tr(g) for g in range(4)]
                return tasks, post

            def attention(c, p, filler):
                nkt = 4 * c + 4
                qTc = qT_t[(c, p)]
                qsl = slice(CH * c, CH * c + CH)
                pso0 = psp.tile([128, CH], F32, tag="pv", bufs=2, name="pso0")
                pso1 = psp.tile([128, CH], F32, tag="pv", bufs=2, name="pso1")
                pend = []   # (kt, ep, off) awaiting PV
                for kt in range(nkt):
                    kTk = kT_c[kt // 4]
                    ks = slice(128 * (kt % 4), 128 * (kt % 4) + 128)
                    dt_ = kt - 4 * c
                    off = 128 * dt_ if dt_ >= 0 else 0
                    sp = psp.tile([128, 2, CH], F32, tag="sp", bufs=2,
                                  name="sp")
                    nc.tensor.matmul(sp[:, 0, off:CH], kTk[0:64, ks],
                                     qTc[0:64, off:CH], start=True, stop=True)
                    nc.tensor.matmul(sp[:, 1, off:CH], kTk[64:128, ks],
                                     qTc[64:128, off:CH], start=True,
                                     stop=True)
                    ep = wp.tile([128, 2, CH], BF, tag="ep", bufs=3, name="ep")
                    nc.scalar.activation(ep[:, :, off:CH], sp[:, :, off:CH],
                                         mybir.ActivationFunctionType.Exp,
                                         scale=0.125)
                    if dt_ >= 0:
                        nc.vector.tensor_tensor(
                            ep[:, :, off:off + 128], ep[:, :, off:off + 128],
                            patd[:], MUL)
                    # drain previous k-tile's PV now (exp of this tile runs on
                    # ACT meanwhile), then interleave filler PE work.  PV is
                    # column-trimmed like the scores: columns below a diagonal
                    # tile's band take no contribution from it.
                    for (pkt, pep, poff) in pend:
                        vv = v_c[pkt // 4][:, pkt % 4, :]
                        nc.tensor.matmul(pso0[:, poff:CH], vv,
                                         pep[:, 0, poff:CH], start=(pkt == 0),
                                         stop=False)
                        nc.tensor.matmul(pso1[:, poff:CH], vv,
                                         pep[:, 1, poff:CH], start=(pkt == 0),
                                         stop=False)
                    pend = [(kt, ep, off)]
                    if filler:
                        filler.pop(0)()
                        if len(filler) % 2 == 0 and filler:
                            filler.pop(0)()
                for (pkt, pep, poff) in pend:
                    vv = v_c[pkt // 4][:, pkt % 4, :]
                    nc.tensor.matmul(pso0[:, poff:CH], vv,
                                     pep[:, 0, poff:CH], start=(pkt == 0),
                                     stop=True)
                    nc.tensor.matmul(pso1[:, poff:CH], vv,
                                     pep[:, 1, poff:CH], start=(pkt == 0),
                                     stop=True)
                for h, pso in ((0, pso0), (1, pso1)):
                    bc = wp.tile([64, CH], F32, tag="bcast", bufs=2, name="bc")
                    nc.vector.tensor_copy(bc[:], pso[HD:2 * HD, :])
                    rc = wp.tile([64, CH], F32, tag="rcp", bufs=2, name="rc")
                    nc.vector.reciprocal_approx_fast(out=rc[:], in_=bc[:])
                    nc.vector.tensor_tensor(
                        attnT[64 * h:64 * h + 64, p, qsl],
                        pso[0:HD, :], rc[:], MUL)

            # ---------------- output projection helpers ----------------
            a2a_sb0 = pp.tile([128, NC_CORES, SC], BF, name="a2a_sb0")
            a2a_sb1 = pp.tile([128, NC_CORES, SC], BF, name="a2a_sb1")
            partials = pp.tile([128, 2 * NCH, CH], BF, name="partials")
            evens = [2 * src for src in range(NC_CORES)]
            odds = [2 * src + 1 for src in range(NC_CORES)]
            chunks = [(qt, nch) for qt in range(2) for nch in range(NCH)]

            def op_mm(psf, qt, nsl, g, start, stop):
                a_ap = (a2a_sb0[:, g // 2, 128 * qt:128 * qt + 128]
                        if g % 2 == 0
                        else a2a_sb1[:, g // 2, 128 * qt:128 * qt + 128])
                nc.tensor.matmul(psf[:], a_ap, wo_sb[:, g, nsl],
                                 start=start, stop=stop)

            ev_psf = {}

            def ev_group(i8, qt, nch):
                nsl = slice(CH * nch, CH * nch + CH)

                def mk(i, g):
                    def f():
                        if i == 0:
                            ev_psf[i8] = psp.tile([128, CH], F32, tag="pj",
                                                  bufs=1, name=f"psfE{i8}")
                        op_mm(ev_psf[i8], qt, nsl, g, i == 0,
                              i == NC_CORES - 1)
                    return f
                fs = [mk(i, g) for i, g in enumerate(evens)]

                def fin():
                    nc.vector.tensor_copy(partials[:, i8, :], ev_psf[i8][:])
                fs.append(fin)
                return fs

            # ---------------- main pipeline (chunk-major) ----------------
            tasks, post = make_proj_tasks(0)
            for t in tasks + post:
                t()
            for c in range(NCH):
                if c + 1 < NCH:
                    filler, post = make_proj_tasks(c + 1)
                else:
                    filler, post = [], []
                attention(c, 0, filler)
                for dst in (2 * c, 2 * c + 1):
                    nc.sync.dma_start(a2a_in0[dst, :, :],
                                      attnT[:, 0, SC * dst:SC * dst + SC])
                # anchored wo prefetch: the 1-element write makes the DMA
                # wait until this point instead of competing with xT early
                nc.vector.tensor_copy(wo_sb[0:1, 4 * c, 0:1],
                                      attnT[0:1, 0, 0:1])
                nc.sync.dma_start(wo_sb[:, 4 * c:4 * c + 4, :],
                                  wo[:, 4 * c:4 * c + 4, :])
                if c == NCH - 1:
                    nc.gpsimd.collective_compute(
                        "AllToAll", mybir.AluOpType.bypass,
                        replica_groups=groups, ins=[a2a_in0.opt()],
                        outs=[a2a_out0.opt()],
                    )
                attention(c, 1, filler)
                for dst in (2 * c, 2 * c + 1):
                    nc.sync.dma_start(a2a_in1[dst, :, :],
                                      attnT[:, 1, SC * dst:SC * dst + SC])
                for t in filler + post:
                    t()
            nc.gpsimd.collective_compute(
                "AllToAll", mybir.AluOpType.bypass,
                replica_groups=groups, ins=[a2a_in1.opt()],
                outs=[a2a_out1.opt()],
            )
            for src in range(NC_CORES):
                nc.sync.dma_start(a2a_sb0[:, src, :], a2a_out0[src, :, :])
            for src in range(NC_CORES):
                nc.sync.dma_start(a2a_sb1[:, src, :], a2a_out1[src, :, :])

            for i8, (qt, nch) in enumerate(chunks):
                psf = psp.tile([128, CH], F32, tag="sp", bufs=2, name="psfE")
                nsl = slice(CH * nch, CH * nch + CH)
                for i, g in enumerate(evens):
                    op_mm(psf, qt, nsl, g, i == 0, i == NC_CORES - 1)
                nc.vector.tensor_copy(partials[:, i8, :], psf[:])
            for i8, (qt, nch) in enumerate(chunks):
                psf = psp.tile([128, CH], F32, tag="sp", bufs=2, name="psfO")
                nsl = slice(CH * nch, CH * nch + CH)
                for i, g in enumerate(odds):
                    op_mm(psf, qt, nsl, g, i == 0, i == NC_CORES - 1)
                osb = wp.tile([128, CH], F32, tag="osb", bufs=2, name="osb")
                nc.vector.tensor_tensor(osb[:], psf[:], partials[:, i8, :],
                                        ADD)
                nc.sync.dma_start(out[128 * qt:128 * qt + 128, nsl], osb[:])

    nc.finalize()
    return nc


def _get_nc():
    if "nc" not in _CACHE:
        _CACHE["nc"] = _build_nc()
    return _CACHE["nc"]


_PERM = np.concatenate([np.arange(0, HD, 2), np.arange(1, HD, 2)])  # de-interleave


def _shard(inputs):
    import ml_dtypes
    BF = ml_dtypes.bfloat16
    x = np.asarray(inputs["x"][0], dtype=np.float32)                 # [S, D]
    # [D, S] -> chunk/partition tiling [NCH, 128, DT, CH] (contiguous DMAs)
    xT = np.ascontiguousarray(
        x.T.astype(BF).reshape(DT, 128, NCH, CH).transpose(2, 1, 0, 3))
    wq = np.asarray(inputs["wq"], dtype=np.float32)
    wk = np.asarray(inputs["wk"], dtype=np.float32)
    wv = np.asarray(inputs["wv"], dtype=np.float32)
    wo = np.ascontiguousarray(
        np.asarray(inputs["wo"]).astype(BF)
        .reshape(DT, 128, DIM).transpose(1, 0, 2))                   # [128,DT,D]
    cos = np.asarray(inputs["freqs_cos"], dtype=np.float32)          # [S, 32]
    sin = np.asarray(inputs["freqs_sin"], dtype=np.float32)
    cosr = np.ascontiguousarray(np.tile(cos.T, (4, 1)).astype(BF))   # [128, S]
    # ev-rows get -sin so rotate_half(x)*sinr lands with the right signs
    sinr = np.ascontiguousarray(
        np.concatenate([-sin.T, sin.T, -sin.T, sin.T], axis=0).astype(BF))
    wq_p = wq.reshape(DIM, 32, HD)[:, :, _PERM]                      # [D,32,64]
    wk_p = wk.reshape(DIM, 8, HD)[:, :, _PERM]
    in_maps = []
    for c in range(NC_CORES):
        q0 = wq_p[:, 4 * c:4 * c + 2, :].reshape(DIM, 128)
        q1 = wq_p[:, 4 * c + 2:4 * c + 4, :].reshape(DIM, 128)
        kc = wk_p[:, c, :]
        vc = wv[:, HD * c:HD * c + HD]
        wqkv = np.ascontiguousarray(
            np.concatenate([q0, q1, kc, vc], axis=1).astype(BF)
            .reshape(DT, 128, WCOLS).transpose(1, 0, 2))             # [128,DT,W]
        in_maps.append({
            "xT": xT,
            "wqkv": wqkv,
            "wo": wo,
            "cosr": cosr,
            "sinr": sinr,
        })
    return in_maps


def kernel(**inputs):
    from concourse.bass_utils import run_bass_kernel_spmd

    nc = _get_nc()
    in_maps = _shard(inputs)
    res = run_bass_kernel_spmd(nc, in_maps, core_ids=list(range(NC_CORES)))
    out = np.concatenate([res.results[c]["out"] for c in range(NC_CORES)],
                         axis=0)
    return out[None].astype(np.float32)
